# revision 60
# baseline (speedup 1.0000x reference)
"""NeuroPlasticLite Trainium2 kernel v3 (8-core data-parallel over batch).

Algorithm: instead of materializing x [B,N,D] each step, track
theta_hat = C1^-t * (DT*L^T g_tilde + c_t*v_hat) per row in 16-dim
Cholesky space (S = w2^T w2 = L L^T), accumulated IN PSUM purely by PE
matmuls (per-step scale C1^-(t+1) folded into 19 precomputed weight
variants).  Norm: ||x_t||^2 = C1^(2t)*(||theta_hat||^2 + kappa_t*rho)
with rho = ||V||^2 - ||v_hat||^2 per row (completed square), so each
step is just: square (Act/DVE) -> 4-level j-tree (DVE) -> newton rsqrt
-> tanh (Act) -> syn matmuls (PE) -> wsel transpose-matmul (PE) -> gelu
(Act) -> 2 accumulation matmuls into theta PSUM (PE).  No per-step x
update, no per-step DMA.

Layouts (per core, batch shard BS=64):
  row = (fc, nlo); fc = bh*64 + nh*32 + bl = m*8 + fcl  (b = bh*32+bl,
  n = nh*128+nlo; chunk m 0..15, fcl 0..7)
  theta PSUM [128 part=nlo, col = m*128 + fcl*16 + j]  f32, 4 banks
  ts / vnt / th0row etc. share that column order
  ts (gelu out) [part = fcl*16+j, col = m-block nlo]  (T-layout)
  batch-half h = bh owns fc [64h, 64h+64) = chunks [8h, 8h+8) —
  CONTIGUOUS columns, giving 2 fully independent pipelines.
"""

import os
from contextlib import ExitStack

import numpy as np

N, D, KF, KN = 256, 32, 16, 50
GAMMA, LAM_A, DT, STEPS = 0.1, 0.95, 0.05, 20
B, UIN = 512, 128
NCORES = 8
BS = B // NCORES
R = BS * N
C1 = 1.0 - DT * GAMMA

SEED_C = 24375.5          # bf16 rsqrt bit-trick affine constant
NDUM = int(os.environ.get("K_NDUM", "3"))   # PE-warm filler matmuls per site
NSQ_FLOOR = 1e-8

_cache = {}


def _gelu(z):
    import math
    erf = np.vectorize(math.erf)
    return 0.5 * z * (1.0 + erf(z / np.sqrt(2.0)))


def _host_prep(features, bias, w_in, b_in, sig_w1, sig_b1, sig_w2, sig_b2):
    import ml_dtypes
    bf = ml_dtypes.bfloat16

    f = features / np.linalg.norm(features, axis=1, keepdims=True)
    sim = f @ f.T
    idx = np.argsort(-sim, axis=1, kind="stable")[:, :KN]
    vals = np.take_along_axis(sim, idx, axis=1)
    W = np.zeros((N, N), np.float32)
    np.add.at(W, (idx, np.arange(N)[:, None]), vals)

    # wt blocks: wt[:, (mh*2+nh)*128 + nlo][mlo] = W[mh*128+mlo, nh*128+nlo]
    wt = np.concatenate(
        [W[mh * 128:(mh + 1) * 128, nh * 128:(nh + 1) * 128]
         for mh in (0, 1) for nh in (0, 1)], axis=1).astype(bf)  # [128, 512]

    w1 = sig_w1[:, 0].astype(np.float32)
    b1 = np.asarray(sig_b1, np.float32)
    w2 = np.asarray(sig_w2, np.float32)            # [D, 16]
    S = w2.T @ w2
    L = np.linalg.cholesky(S)                      # [16,16]
    LinvT = np.linalg.inv(L).T
    w2LT = (w2 @ LinvT).astype(np.float32)         # [D, 16]

    badd = (bias + b_in[None, :] + sig_b2[None, :]).astype(np.float32)  # [N,D]

    # wselBig [128, 2048]: [p, m*128 + fcl*16 + j] = w1[j] * (p == m*8+fcl)
    wselBig = np.zeros((128, 2048), np.float32)
    for m in range(16):
        for fcl in range(8):
            wselBig[m * 8 + fcl, m * 128 + fcl * 16:m * 128 + fcl * 16 + 16] = w1
    wselBig = wselBig.astype(bf)

    b1T = np.asarray([b1[q % 16] for q in range(128)], np.float32)[:, None]

    # Lhat [128, 20*128]: block t (t=0..19): rows (fcl,j), cols (fcl',j'):
    #   delta(fcl,fcl') * DT * C1^-(t+1) * L[j, j']
    Lhat = np.zeros((128, 20 * 128), np.float32)
    for t in range(STEPS):
        c = DT * (C1 ** (-(t + 1)))
        blk = np.zeros((128, 128), np.float32)
        for fcl in range(8):
            blk[fcl * 16:(fcl + 1) * 16, fcl * 16:(fcl + 1) * 16] = c * L
        Lhat[:, t * 128:(t + 1) * 128] = blk
    Lhat = Lhat.astype(bf)

    # ident scale factors (identsBig built on-device from identc20):
    #   k=0..19: C1^-(k+1) (v-add step k); k=20: -c20*C1^-20 (final vsub)
    csum = np.zeros(STEPS + 1, np.float64)   # csum[t] = c_t = sum_{s<t} C1^s
    for t in range(1, STEPS + 1):
        csum[t] = 1.0 + C1 * csum[t - 1]
    c20 = float(csum[STEPS])
    iscales = [float((C1 ** (-(k + 1))) / c20) for k in range(20)]
    iscales.append(float(-(C1 ** (-STEPS))))
    eye = np.eye(128, dtype=np.float32)

    # G0col [128,1]: gelu(b1)[j] at partition fcl*16+j (for theta init
    # via the Lhat t=0 block applied to a G0-broadcast ts)
    G0 = _gelu(b1).astype(np.float32)
    G0col = np.asarray([G0[q % 16] for q in range(128)],
                       np.float32)[:, None].astype(bf)

    Wv = (DT * (w_in.T @ w2LT)).astype(bf)                # [UIN=128, 16]
    # bvn [128, 2048] bf16 (nt layout): [nlo, m*128+fcl*16+j] =
    #   DT * (badd @ w2LT)[nh(m)*128 + nlo, j]
    bw = DT * (badd @ w2LT)                               # [N, 16]
    bvn = np.zeros((128, 2048), np.float32)
    for m in range(16):
        for fcl in range(8):
            nh = ((m * 8 + fcl) // 32) % 2
            bvn[:, m * 128 + fcl * 16:m * 128 + (fcl + 1) * 16] = \
                bw[nh * 128:(nh + 1) * 128, :]
    bvn = bvn.astype(bf)

    winTb = (DT * w_in.T).astype(bf)                      # [128, 32]
    # biasV [128, 64] f32: [nlo, nh*32+d] = DT*badd[nh*128+nlo, d]
    biasV = np.concatenate(
        [DT * badd[0:128, :], DT * badd[128:256, :]], axis=1).astype(np.float32)

    # bdL [128, 256]: [fcl*16+j, fcl*32+d] = C1^20 * w2LT[d, j]
    bdL = np.zeros((128, 256), np.float32)
    cf = C1 ** STEPS
    for fcl in range(8):
        bdL[fcl * 16:(fcl + 1) * 16, fcl * 32:(fcl + 1) * 32] = cf * w2LT.T
    bdL = bdL.astype(bf)

    identc20 = (csum[STEPS] * eye).astype(bf)

    kappas = [float((C1 ** (-2 * t)) * (csum[t] ** 2)) for t in range(STEPS)]
    tscales = [float(C1 ** t) for t in range(STEPS)]
    sq2scales = [float(C1 ** (2 * t)) for t in range(STEPS)]

    packF32 = np.concatenate([b1T, biasV], axis=1)        # [128, 65] f32
    packT = np.concatenate([winTb, Wv, G0col, identc20],
                           axis=1)                        # [128, 177] bf16
    packV = np.concatenate([bvn, Lhat[:, 0:128]],
                           axis=1)                        # [128, 2176] bf16
    packB = np.concatenate([Lhat, wselBig, wt, bdL],
                           axis=1)                        # [128, 5376] bf16
    return dict(packF32=packF32, packT=packT, packV=packV, packB=packB,
                iscales=iscales,
                kappas=kappas, tscales=tscales, sq2scales=sq2scales)


def build_nc(hp, n_cores):
    import concourse.bacc as bacc
    import concourse.tile as tile
    from concourse import mybir

    f32 = mybir.dt.float32
    bf16 = mybir.dt.bfloat16
    i16 = mybir.dt.int16
    AF = mybir.ActivationFunctionType
    OP = mybir.AluOpType
    AX = mybir.AxisListType

    iscales = hp["iscales"]
    kappas = hp["kappas"]
    tscales = hp["tscales"]
    sq2scales = hp["sq2scales"]

    nc = bacc.Bacc("TRN2", target_bir_lowering=False, debug=False,
                   num_devices=n_cores)
    uT_d = nc.declare_dram_parameter("uT", [128, R], bf16, isOutput=False)
    dram = {}
    for nm, shp, dt in [
            ("packF32", [128, 65], f32), ("packT", [128, 177], bf16),
            ("packV", [128, 2176], bf16), ("packB", [128, 5376], bf16)]:
        dram[nm] = nc.declare_dram_parameter(nm, shp, dt, isOutput=False)
    xout = nc.declare_dram_parameter("xout", [128, 4096], bf16, isOutput=True)

    # half h columns of theta/ts space: chunks {4h..4h+3, 8+4h..8+4h+3}
    def hruns(h, w):
        # two runs of width 4*w starting at cols h*4*w and 8*w + h*4*w
        return [(h * 4 * w, 4 * w), ((8 + 4 * h) * w, 4 * w)]

    with tile.TileContext(nc) as tc:
        with ExitStack() as ctx:
            cpool = ctx.enter_context(tc.tile_pool(name="consts", bufs=1))
            packF32 = cpool.tile([128, 65], f32, name="packF32")
            packT = cpool.tile([128, 177], bf16, name="packT")
            packV = cpool.tile([128, 2176], bf16, name="packV")
            packB = cpool.tile([128, 5376], bf16, name="packB")
            ct = {
                "b1T": packF32[:, 0:1],
                "biasV": packF32[:, 1:65],
                "winTb": packT[:, 0:32],
                "Wv": packT[:, 32:48],
                "G0col": packT[:, 48:49],
                "identc20": packT[:, 49:177],
                "bvn": packV[:, 0:2048],
                "Lhat0": packV[:, 2048:2176],
                "Lhat": packB[:, 0:2560],
                "wselBig": packB[:, 2560:4608],
                "wt": packB[:, 4608:5120],
                "bdL": packB[:, 5120:5376],
            }

            spool = ctx.enter_context(tc.tile_pool(name="state", bufs=1))
            uTs = spool.tile([128, 16384], bf16)
            V_bf = spool.tile([128, 4096], bf16)
            vnt = spool.tile([128, 2048], bf16)
            rho = spool.tile([128, 128], f32)
            synT = [spool.tile([128, 128], bf16, name=f"synT{i}")
                    for i in range(2)]
            ts = spool.tile([128, 2048], bf16)
            thc = spool.tile([128, 2048], bf16)
            thT = spool.tile([128, 2048], bf16)
            krbig = spool.tile([128, 19 * 128], bf16)
            identsBig = spool.tile([128, 21 * 128], bf16)
            xsb = spool.tile([128, 4096], bf16)

            # persistent PSUM for theta_hat (4 banks)
            thpool = ctx.enter_context(
                tc.tile_pool(name="thp", bufs=1, space="PSUM"))
            thp = thpool.tile([128, 2048], f32)

            nc.gpsimd.memset(synT[0][:], 0)
            nc.gpsimd.memset(synT[1][:], 0)

            # ---------------- Phase A ----------------
            nc.sync.dma_start(packF32[:], dram["packF32"][:])
            nc.sync.dma_start(packT[:], dram["packT"][:])

            with ExitStack() as actx:
                app = actx.enter_context(
                    tc.tile_pool(name="apsum", bufs=2, space="PSUM"))
                apool = actx.enter_context(tc.tile_pool(name="asb", bufs=2))

                nc.sync.dma_start(uTs[:, 0:4096], uT_d[:, 0:4096])
                nc.sync.dma_start(packV[:], dram["packV"][:])
                for q in range(1, 4):
                    nc.sync.dma_start(uTs[:, q * 4096:(q + 1) * 4096],
                                      uT_d[:, q * 4096:(q + 1) * 4096])
                nc.sync.dma_start(packB[:], dram["packB"][:])
                nc.gpsimd.tensor_scalar(
                    identsBig[:, 0:128], ct["identc20"], iscales[0], 0.0,
                    op0=OP.mult, op1=OP.add)
                g0b = ct["G0col"].broadcast_to((128, 2048))
                nc.gpsimd.tensor_copy(ts[:], g0b)

                pp = apool.tile([128, 128], f32, tag="pp", name="pp")
                vv = apool.tile([128, 128], f32, tag="vv", name="vv")
                for q in range(4):
                    # v_hat (nt layout): 32 mm of 16 cols
                    vq = app.tile([128, 512], f32, tag="vq", name="vq")
                    for sidx in range(32):
                        fc = q * 32 + sidx
                        nc.tensor.matmul(
                            vq[:, sidx * 16:(sidx + 1) * 16],
                            uTs[:, fc * 128:(fc + 1) * 128], ct["Wv"],
                            start=True, stop=True, skip_group_check=True)
                    nc.vector.tensor_tensor(
                        vnt[:, q * 512:(q + 1) * 512], vq[:],
                        ct["bvn"][:, q * 512:(q + 1) * 512], op=OP.add)
                    # u_proj -> V_bf: 32 mm of 32 cols (2 psum tiles of 512)
                    nh = q % 2
                    for half in range(2):
                        vp = app.tile([128, 512], f32, tag="vp", name="vp")
                        for sidx in range(16):
                            fc = q * 32 + half * 16 + sidx
                            nc.tensor.matmul(
                                vp[:, sidx * 32:(sidx + 1) * 32],
                                uTs[:, fc * 128:(fc + 1) * 128],
                                ct["winTb"],
                                start=True, stop=True, skip_group_check=True)
                        vv3 = vp[:].rearrange("p (s d) -> p s d", d=32)
                        bV = ct["biasV"][:, nh * 32:(nh + 1) * 32]\
                            .unsqueeze(1).broadcast_to((128, 16, 32))
                        Vv = V_bf[:, (q * 32 + half * 16) * 32:
                                  (q * 32 + half * 16 + 16) * 32]\
                            .rearrange("p (s d) -> p s d", d=32)
                        nc.vector.tensor_tensor(Vv, vv3, bV, op=OP.add)
                    # pp-q: ||V||^2 over d=32 (square + reduce)
                    psq = apool.tile([128, 1024], bf16, tag="psq", name="psq")
                    if q < 2:
                        vbs = V_bf[:, q * 1024:(q + 1) * 1024]
                        nc.vector.tensor_tensor(psq[:], vbs, vbs, op=OP.mult)
                    else:
                        nc.scalar.activation(
                            psq[:], V_bf[:, q * 1024:(q + 1) * 1024],
                            AF.Square)
                    if q < 2:
                        # Pool tree (5 levels) in pt scratch [128, 1024]
                        pt = apool.tile([128, 1024], bf16, tag="pt",
                                        name="pt")
                        a0 = psq[:].rearrange("p (f d) -> p f d", d=32)
                        o1 = pt[:, 0:512].rearrange("p (f d) -> p f d", d=16)
                        nc.gpsimd.tensor_tensor(o1, a0[:, :, 0:16],
                                                a0[:, :, 16:32], op=OP.add)
                        a1 = pt[:, 0:512].rearrange("p (f d) -> p f d", d=16)
                        o2 = pt[:, 512:768].rearrange("p (f d) -> p f d",
                                                      d=8)
                        nc.gpsimd.tensor_tensor(o2, a1[:, :, 0:8],
                                                a1[:, :, 8:16], op=OP.add)
                        a2 = pt[:, 512:768].rearrange("p (f d) -> p f d",
                                                      d=8)
                        o3 = pt[:, 768:896].rearrange("p (f d) -> p f d",
                                                      d=4)
                        nc.gpsimd.tensor_tensor(o3, a2[:, :, 0:4],
                                                a2[:, :, 4:8], op=OP.add)
                        a3 = pt[:, 768:896].rearrange("p (f d) -> p f d",
                                                      d=4)
                        o4 = pt[:, 896:960].rearrange("p (f d) -> p f d",
                                                      d=2)
                        nc.gpsimd.tensor_tensor(o4, a3[:, :, 0:2],
                                                a3[:, :, 2:4], op=OP.add)
                        a4 = pt[:, 896:960].rearrange("p (f d) -> p f d",
                                                      d=2)
                        nc.gpsimd.tensor_tensor(
                            pp[:, q * 32:(q + 1) * 32], a4[:, :, 0],
                            a4[:, :, 1], op=OP.add)
                    else:
                        pv = psq[:].rearrange("p (f d) -> p f d", d=32)
                        nc.vector.tensor_reduce(pp[:, q * 32:(q + 1) * 32],
                                                pv, axis=AX.X, op=OP.add)
                    # vv-q: ||v_hat||^2 over j=16
                    vsq = apool.tile([128, 512], bf16, tag="vsq", name="vsq")
                    nc.scalar.activation(
                        vsq[:], vnt[:, q * 512:(q + 1) * 512], AF.Square)
                    sv = vsq[:].rearrange("p (f j) -> p f j", j=16)
                    nc.vector.tensor_reduce(vv[:, q * 32:(q + 1) * 32], sv,
                                            axis=AX.X, op=OP.add)
                # rho = max(pp - vv, 0)
                for q in range(4):
                    nc.vector.scalar_tensor_tensor(
                        rho[:, q * 32:(q + 1) * 32],
                        vv[:, q * 32:(q + 1) * 32], -1.0,
                        pp[:, q * 32:(q + 1) * 32],
                        op0=OP.mult, op1=OP.add)
                    nc.vector.tensor_scalar(
                        rho[:, q * 32:(q + 1) * 32],
                        rho[:, q * 32:(q + 1) * 32], 0.0, 0.0,
                        op0=OP.max, op1=OP.add)
                # theta init: Lhat block 0 applied to G0-filled ts + v-add
                for q in range(4):
                    for m in range(q * 4, q * 4 + 4):
                        nc.tensor.matmul(
                            thp[:, m * 128:(m + 1) * 128],
                            ts[:, m * 128:(m + 1) * 128],
                            ct["Lhat0"],
                            start=True, stop=False, skip_group_check=True)
                    sl = slice(q * 512, (q + 1) * 512)
                    nc.tensor.matmul(
                        thp[:, sl], identsBig[:, 0:128], vnt[:, sl],
                        start=False, stop=False, skip_group_check=True)
                # kr_1 only; later steps fill kr_{t+1} in-loop on Pool
                nc.scalar.activation(krbig[:, 0:128], rho[:],
                                     AF.Copy, scale=kappas[1])
                # remaining ident blocks (Pool, off the critical path)
                for k in range(1, 21):
                    nc.gpsimd.tensor_scalar(
                        identsBig[:, k * 128:(k + 1) * 128],
                        ct["identc20"], iscales[k], 0.0,
                        op0=OP.mult, op1=OP.add)

            # ---------------- loop pools ----------------
            lp = ctx.enter_context(tc.tile_pool(name="loop", bufs=2))
            sypool = ctx.enter_context(
                tc.tile_pool(name="syp", bufs=1, space="PSUM"))
            dumpool = ctx.enter_context(
                tc.tile_pool(name="dum", bufs=1, space="PSUM"))
            tspool = ctx.enter_context(
                tc.tile_pool(name="tsp", bufs=2, space="PSUM"))

            # quarter q2 = h*2+g owns theta cols [q2*512,(q2+1)*512),
            # norm cols [q2*32,(q2+1)*32).
            # q2 odd: Act-square + DVE TensorReduce; q2 even: DVE-copy +
            # Pool TT-square + Pool tree (keeps Act/DVE free).
            def norm_q(t, q2, T):
                sl = slice(q2 * 512, (q2 + 1) * 512)
                nsl = slice(q2 * 32, (q2 + 1) * 32)
                kr = krbig[:, (t - 1) * 128 + q2 * 32:
                           (t - 1) * 128 + (q2 + 1) * 32]
                nc.scalar.activation(T["sq"][:, sl], thp[:, sl], AF.Square)
                sv = T["sq"][:, sl].rearrange("p (f j) -> p f j", j=16)
                nc.vector.tensor_reduce(T["t4"][:, nsl], sv,
                                        axis=AX.X, op=OP.add)
                nc.vector.tensor_tensor(T["nsq"][:, nsl], T["t4"][:, nsl],
                                        kr, op=OP.add)

            def st_stage(t, h, T):
                sl = slice(h * 64, (h + 1) * 64)
                nc.vector.tensor_scalar(
                    T["y0"][:, sl].bitcast(i16), T["nsq"][:, sl].bitcast(i16),
                    -0.5, SEED_C, op0=OP.mult, op1=OP.add)
                nc.vector.tensor_tensor(
                    T["p1"][:, sl], T["nsq"][:, sl], T["y0"][:, sl],
                    op=OP.mult)
                nc.gpsimd.tensor_tensor(
                    T["p2"][:, sl], T["p1"][:, sl], T["y0"][:, sl],
                    op=OP.mult)
                nc.gpsimd.tensor_scalar(
                    T["ww"][:, sl], T["p2"][:, sl], -0.5, 1.5,
                    op0=OP.mult, op1=OP.add)
                nc.vector.tensor_tensor(
                    T["nrm"][:, sl], T["p1"][:, sl], T["ww"][:, sl],
                    op=OP.mult)
                nc.scalar.activation(
                    T["a"][:, sl], T["nrm"][:, sl], AF.Tanh,
                    scale=tscales[t])

            def syn_stage(t, h, T):
                syp = T["syp"][:, h * 128:(h + 1) * 128]
                for nh in (0, 1):
                    for mh in (0, 1):
                        nc.tensor.matmul(
                            syp[h * 64 + nh * 32:h * 64 + nh * 32 + 32, :],
                            T["a"][:, h * 64 + mh * 32:h * 64 + mh * 32 + 32],
                            ct["wt"][:, (mh * 2 + nh) * 128:
                                     (mh * 2 + nh + 1) * 128],
                            start=(mh == 0), stop=(mh == 1),
                            skip_group_check=True,
                            tile_position=(0, h * 64 + nh * 32))

            def cp_stage(t, h, T):
                syp = T["syp"][:, h * 128:(h + 1) * 128]
                pr = slice(h * 64, h * 64 + 64)
                nc.vector.tensor_copy(synT[h][pr, :], syp[pr, :])

            def acc_group(t, h, g):
                lb = t * 128
                for m in range(8 * h + 4 * g, 8 * h + 4 * g + 4):
                    nc.tensor.matmul(
                        thp[:, m * 128:(m + 1) * 128],
                        ts[:, m * 128:(m + 1) * 128],
                        ct["Lhat"][:, lb:lb + 128],
                        start=False, stop=False, skip_group_check=True)

            def vadd_q(t, q2):
                sl = slice(q2 * 512, (q2 + 1) * 512)
                nc.tensor.matmul(
                    thp[:, sl], identsBig[:, t * 128:(t + 1) * 128],
                    vnt[:, sl],
                    start=False, stop=False, skip_group_check=True)

            def wsel_gelu_one(t, h, g, T):
                tsp = tspool.tile([128, 512], f32, tag="tsps", name="tsp")
                for k in range(4):
                    m = 8 * h + 4 * g + k
                    nc.tensor.matmul(
                        tsp[:, k * 128:(k + 1) * 128],
                        ct["wselBig"][:, m * 128:(m + 1) * 128],
                        synT[h][:, :],
                        start=True, stop=True, skip_group_check=True)
                base = (8 * h + 4 * g) * 128
                nc.scalar.activation(
                    ts[:, base:base + 512], tsp[:], AF.Gelu,
                    bias=ct["b1T"])

            def dummy_mm(n=1):
                for _ in range(n):
                    dp = dumpool.tile([128, 512], f32, tag="dum", name="dum")
                    nc.tensor.matmul(
                        dp[:], ct["identc20"],
                        ct["wselBig"][:, 0:512],
                        start=True, stop=True, skip_group_check=True)

            def final_q(q2):
                # inline part: vsub -> thc copy (DVE) -> transpose
                sl = slice(q2 * 512, (q2 + 1) * 512)
                nc.tensor.matmul(thp[:, sl],
                                 identsBig[:, 20 * 128:21 * 128],
                                 vnt[:, sl], start=False, stop=True,
                                 skip_group_check=True)
                nc.vector.tensor_copy(thc[:, sl], thp[:, sl])
                tout = thT[:, q2 * 512:(q2 + 1) * 512]\
                    .rearrange("p (m n) -> p m n", n=128)
                nc.sync.dma_start(tout, thc[:, q2 * 512:(q2 + 1) * 512],
                                  transpose=True)

            def final_out():
                # post-loop: x matmuls -> per-quarter sbuf copies -> 2 DMAs
                pools = [("tsps", tspool), ("dum", dumpool)]
                for q2 in range(4):
                    # two 512-col psum tiles per quarter
                    for xb in range(2):
                        tg, pl = pools[(q2 * 2 + xb) % 2]
                        xqt = pl.tile([128, 512], f32, tag=tg, name="xqt")
                        for k in range(2):
                            m = q2 * 4 + xb * 2 + k
                            nc.tensor.matmul(
                                xqt[:, k * 256:(k + 1) * 256],
                                ct["identc20"],
                                V_bf[:, m * 256:(m + 1) * 256],
                                start=True, stop=False,
                                skip_group_check=True)
                            nc.tensor.matmul(
                                xqt[:, k * 256:(k + 1) * 256],
                                thT[:, m * 128:(m + 1) * 128], ct["bdL"],
                                start=False, stop=True,
                                skip_group_check=True)
                        osl = slice(q2 * 1024 + xb * 512,
                                    q2 * 1024 + (xb + 1) * 512)
                        if q2 == 2:
                            nc.scalar.copy(xsb[:, osl], xqt[:])
                        else:
                            nc.vector.tensor_copy(xsb[:, osl], xqt[:])
                    if q2 == 1:
                        nc.sync.dma_start(xout[:, 0:2048], xsb[:, 0:2048])
                    elif q2 == 2:
                        nc.sync.dma_start(xout[:, 2048:3072],
                                          xsb[:, 2048:3072])
                nc.sync.dma_start(xout[:, 3072:3584], xsb[:, 3072:3584])
                nc.sync.dma_start(xout[:, 3584:4096], xsb[:, 3584:4096])

            def make_T():
                T = {}
                T["sq"] = lp.tile([128, 2048], bf16, tag="sq", name="sq")
                T["t4"] = lp.tile([128, 128], f32, tag="t4", name="t4")
                T["nsq"] = lp.tile([128, 128], bf16, tag="nsq", name="nsq")
                T["y0"] = lp.tile([128, 128], bf16, tag="y0", name="y0")
                T["p1"] = lp.tile([128, 128], f32, tag="p1", name="p1")
                T["p2"] = lp.tile([128, 128], bf16, tag="p2", name="p2")
                T["ww"] = lp.tile([128, 128], bf16, tag="ww", name="ww")
                T["nrm"] = lp.tile([128, 128], f32, tag="nrm", name="nrm")
                T["a"] = lp.tile([128, 128], bf16, tag="a", name="a")
                T["syp"] = sypool.tile([128, 256], f32, tag="sy", name="syp")
                return T

            # prologue: norm-front of step 1
            T = make_T()
            for q2 in (0, 1, 2, 3):
                norm_q(1, q2, T)
            for t in range(1, STEPS):
                Tn = make_T() if t < STEPS - 1 else None
                if t < STEPS - 1:
                    nc.gpsimd.tensor_scalar(
                        krbig[:, t * 128:(t + 1) * 128], rho[:],
                        kappas[t + 1], 0.0, op0=OP.mult, op1=OP.add)
                st_stage(t, 0, T)
                syn_stage(t, 0, T)
                cp_stage(t, 0, T)
                dummy_mm(NDUM)
                st_stage(t, 1, T)
                syn_stage(t, 1, T)
                cp_stage(t, 1, T)
                for h, g in ((0, 0), (0, 1), (1, 0), (1, 1)):
                    q2 = 2 * h + g
                    wsel_gelu_one(t, h, g, T)
                    acc_group(t, h, g)
                    vadd_q(t, q2)
                    if t < STEPS - 1:
                        dummy_mm(NDUM)
                        norm_q(t + 1, q2, Tn)
                    else:
                        final_q(q2)
                T = Tn
            final_out()

    nc.finalize()
    return nc


def _get_nc(key, hp, n_cores):
    if key not in _cache:
        _cache[key] = build_nc(hp, n_cores)
    return _cache[key]


def kernel(u, features, bias, w_in, b_in, sig_w1, sig_b1, sig_w2, sig_b2):
    import ml_dtypes
    from concourse.bass_utils import run_bass_kernel_spmd

    bf = ml_dtypes.bfloat16
    u = np.asarray(u, np.float32)
    args = [np.asarray(a, np.float32) for a in
            (features, bias, w_in, b_in, sig_w1, sig_b1, sig_w2, sig_b2)]
    hp = _host_prep(*args)

    key = "v3"
    nc = _get_nc(key, hp, NCORES)

    in_maps = []
    for c in range(NCORES):
        # [bh, bl, nh, nlo, k] -> [k, bh, nh, bl, nlo]
        ush = u[c * BS:(c + 1) * BS].reshape(2, 32, 2, 128, UIN)
        uT = np.ascontiguousarray(
            ush.transpose(4, 0, 2, 1, 3).reshape(UIN, R)).astype(bf)
        m = {"uT": uT}
        for k in ("packF32", "packT", "packV", "packB"):
            m[k] = hp[k]
        in_maps.append(m)

    kw = {}
    if os.environ.get("KERNEL_TRACE"):
        td = os.environ.get("KERNEL_TRACE_DIR")
        if td:
            os.makedirs(td, exist_ok=True)
        kw = dict(trace=True, tmpdir=td)
    res = run_bass_kernel_spmd(nc, in_maps, list(range(NCORES)), **kw)
    if os.environ.get("KERNEL_TRACE"):
        globals()["_last_hw_ns"] = res.exec_time_ns
        globals()["_last_trace"] = res.instructions_and_trace
        globals()["_last_profile_json"] = res.profile_json

    out = np.empty((B, N, D), np.float32)
    for c in range(NCORES):
        xo = np.asarray(res.results[c]["xout"], np.float32)   # [128, 4096]
        v = xo.reshape(128, 2, 2, 32, 32)             # [nlo, bh, nh, bl, d]
        out[c * BS:(c + 1) * BS] = \
            v.transpose(1, 3, 2, 0, 4).reshape(BS, N, D)
    return out


_last_sim_ns = 184835  # updated after TimelineSim measurement
